# revision 30
# baseline (speedup 1.0000x reference)
"""AdaAffcell kernel for 8 TRN2 NeuronCores (data-parallel over batch).

Math per row b (reference):
    hx    = [h | x]                               (256 features)
    z     = hx @ W_a^T + b_a                      (128)
    g     = hx @ W_g^T + b_g                      (128)
    alpha = sigmoid(g)
    ht    = alpha * tanh(z) + (1 - alpha) * z
    out   = LayerNorm(ht) * gamma + beta          (gamma==1, beta==0 in setup)

Design (per core, 65536 rows):
  - Host pre-transposes x/h to feature-major [128, rows] bf16; weights,
    biases replicated and bf16.  Host un-permutes the bf16 output.
  - Super-tiles of 4096 rows: ONE 1MB dma_start per input per super-tile
    (descriptor-rate matters: >=1MB per DMA for efficiency), split across
    both HWDGE rings (sync + scalar).
  - Per 128-row block, PSUM [128, 256] = [z|g]: bias matmul (K=1, N=512,
    seeds 2 blocks) + two K=128 bf16 matmuls per block.
  - ScalarE: tanh(z), sigmoid(g), z-copy (PSUM -> SBUF bf16).
  - VectorE: blend (u = t-z; v = s*u; ht = v+z) bf16 2x, per-block
    bn_stats, batched stats massage.
  - GpSimdE: fused (ht*rstd - mu*rstd) per-block normalize.
  - Output written tile-contiguous (1MB per dma_start), host un-permutes.
"""

import sys

sys.path.insert(0, "/opt/trn_rl_repo")

import numpy as np
import ml_dtypes
from contextlib import ExitStack

import concourse.bass as bass
import concourse.tile as tile
from concourse import mybir
from concourse.bass_utils import run_bass_kernel_spmd

BF16_NP = ml_dtypes.bfloat16

N_CORES = 8
B = 524288
ROWS = B // N_CORES          # 65536 rows per core
T = 8                        # 128-row blocks per psum round
RPS = 4                      # psum rounds per super-tile
SUP = RPS * T * 128          # 4096 rows per super-tile
N_SUP = ROWS // SUP          # 16
EPS = 1e-5

F32 = mybir.dt.float32
BF16 = mybir.dt.bfloat16

_BUILD_CACHE = {}


def build_bass(loop_n=None, ablate=None):
    nc = bass.Bass()
    hT = nc.declare_dram_parameter("hT", [128, ROWS], BF16, isOutput=False)
    xT = nc.declare_dram_parameter("xT", [128, ROWS], BF16, isOutput=False)
    wt0 = nc.declare_dram_parameter("wt0", [128, 256], BF16, isOutput=False)
    wt1 = nc.declare_dram_parameter("wt1", [128, 256], BF16, isOutput=False)
    bzg = nc.declare_dram_parameter("bzg", [1, 512], BF16, isOutput=False)
    onesd = nc.declare_dram_parameter("onesd", [1, 128], BF16, isOutput=False)
    out = nc.declare_dram_parameter("out", [N_SUP * 128, SUP], BF16,
                                    isOutput=True)

    with tile.TileContext(nc) as tc, ExitStack() as ctx:
        singles = ctx.enter_context(tc.tile_pool(name="singles", bufs=1))
        loads = ctx.enter_context(tc.tile_pool(name="loads", bufs=3))
        mids = ctx.enter_context(tc.tile_pool(name="mids", bufs=3))
        psums = ctx.enter_context(tc.tile_pool(name="psum", bufs=2,
                                               space="PSUM"))
        outs = ctx.enter_context(tc.tile_pool(name="outs", bufs=3))

        w0 = singles.tile([128, 256], BF16)
        w1 = singles.tile([128, 256], BF16)
        bz2 = singles.tile([1, 512], BF16)
        ones = singles.tile([1, 128], BF16)
        epsb = singles.tile([128, 1], F32)
        QTC = RPS * T
        magic_i = singles.tile([128, QTC, 1], mybir.dt.int32)
        one_i = singles.tile([128, QTC, 1], mybir.dt.int32)
        nc.sync.dma_start(out=w0[:], in_=wt0[:])
        nc.sync.dma_start(out=w1[:], in_=wt1[:])
        nc.sync.dma_start(out=bz2[:], in_=bzg[:])
        nc.sync.dma_start(out=ones[:], in_=onesd[:])
        nc.vector.memset(epsb[:], EPS)
        nc.vector.memset(magic_i[:], 0x5F3759DF)
        nc.vector.memset(one_i[:], 1)

        import contextlib
        loop_cm = tc.For_i(0, loop_n, 1) if loop_n else contextlib.nullcontext()
        with loop_cm:
          for sti in range(N_SUP):
            r0 = sti * SUP
            h_s = loads.tile([128, SUP], BF16, tag="hT")
            x_s = loads.tile([128, SUP], BF16, tag="xT")
            nc.sync.dma_start(out=h_s[:], in_=hT[:, r0:r0 + SUP])
            nc.sync.dma_start(out=x_s[:], in_=xT[:, r0:r0 + SUP])

            o_s = outs.tile([128, RPS * T, 128], BF16, tag="o")
            ht_s = outs.tile([128, RPS * T, 128], BF16, tag="hts")
            st = outs.tile([128, RPS * T, 6], F32, tag="st")
            out_ap = out[sti * 128:(sti + 1) * 128, :].rearrange(
                "p (q f) -> p q f", q=RPS * T)

            if ablate == "dma":
                nc.sync.dma_start(
                    out=out_ap,
                    in_=h_s[:].rearrange("p (q f) -> p q f", q=RPS * T))
                continue

            for rnd in range(RPS):
                base = rnd * T * 128
                ps = psums.tile([128, T, 256], F32, tag="ps")
                for j in range(0, T, 2):
                    # one N=512 bias matmul seeds two blocks
                    nc.tensor.matmul(
                        ps[:, j:j + 2, :], ones[:], bz2[:],
                        start=True, stop=False,
                    )
                    for jj in (j, j + 1):
                        c0 = base + jj * 128
                        nc.tensor.matmul(
                            ps[:, jj, :], h_s[:, c0:c0 + 128], w0[:],
                            start=False, stop=False,
                        )
                        nc.tensor.matmul(
                            ps[:, jj, :], x_s[:, c0:c0 + 128], w1[:],
                            start=False, stop=True,
                        )

                z_ap = ps[:, :, 0:128]     # z + b_a
                g_ap = ps[:, :, 128:256]   # g + b_g
                osl = o_s[:, rnd * T:(rnd + 1) * T, :]

                if ablate in ("mm1", "noveq"):
                    if ablate == "noveq":
                        t_t = mids.tile([128, T, 128], BF16, tag="t")
                        s_t = mids.tile([128, T, 128], BF16, tag="s")
                        nc.scalar.activation(
                            t_t[:], z_ap, mybir.ActivationFunctionType.Tanh)
                        nc.scalar.activation(
                            s_t[:], g_ap, mybir.ActivationFunctionType.Sigmoid)
                    nc.scalar.copy(osl, z_ap)
                    continue

                t_t = mids.tile([128, T, 128], BF16, tag="t")
                s_t = mids.tile([128, T, 128], BF16, tag="s")
                zc = mids.tile([128, T, 128], BF16, tag="zc")
                nc.scalar.activation(t_t[:], z_ap,
                                     mybir.ActivationFunctionType.Tanh)
                nc.scalar.activation(s_t[:], g_ap,
                                     mybir.ActivationFunctionType.Sigmoid)
                nc.scalar.copy(zc[:], z_ap)

                u = mids.tile([128, T, 128], BF16, tag="u")
                v = mids.tile([128, T, 128], BF16, tag="v")
                htl = ht_s[:, rnd * T:(rnd + 1) * T, :]
                nc.vector.tensor_sub(u[:], t_t[:], zc[:])
                nc.vector.tensor_mul(v[:], s_t[:], u[:])
                nc.vector.tensor_add(htl, v[:], zc[:])

                # Per-row stats: HW bn_stats emits exactly 6 vals/partition.
                for j in range(T):
                    nc.vector.bn_stats(st[:, rnd * T + j, :],
                                       ht_s[:, rnd * T + j, :])

            # Stats finalize once per super-tile: ONE Sqrt keeps ACT
            # table-set thrash (sigmoid-set <-> sqrt-set) to 2 loads per
            # super-tile instead of 2 per round.
            # mean = (m_e+m_o)/2; var = (cv_e+cv_o)/128 + (m_e-m_o)^2/4
            QT = RPS * T
            m_e, cv_e = st[:, :, 1:2], st[:, :, 2:3]
            m_o, cv_o = st[:, :, 4:5], st[:, :, 5:6]
            msum = mids.tile([128, QT, 1], F32, tag="msum")
            d = mids.tile([128, QT, 1], F32, tag="d")
            dh = mids.tile([128, QT, 1], F32, tag="dh")
            cv = mids.tile([128, QT, 1], F32, tag="cv")
            d2 = mids.tile([128, QT, 1], F32, tag="d2")
            var = mids.tile([128, QT, 1], F32, tag="var")
            rs = mids.tile([128, QT, 1], F32, tag="rs")
            nb = mids.tile([128, QT, 1], F32, tag="nb")
            nc.vector.tensor_add(msum[:], m_e, m_o)
            nc.vector.tensor_sub(d[:], m_e, m_o)
            nc.vector.tensor_scalar_mul(dh[:], d[:], 0.5)
            nc.vector.tensor_add(cv[:], cv_e, cv_o)
            nc.vector.tensor_mul(d2[:], dh[:], dh[:])
            nc.vector.scalar_tensor_tensor(
                var[:], cv[:], 1.0 / 128.0, d2[:],
                op0=mybir.AluOpType.mult, op1=mybir.AluOpType.add,
            )
            # rs = rsqrt(var + eps) via bit-trick seed + 2 Newton steps —
            # keeps Sqrt off ScalarE (ACT table-set thrash with sigmoid set).
            vare = mids.tile([128, QT, 1], F32, tag="vare")
            xh = mids.tile([128, QT, 1], F32, tag="xh")
            yb = mids.tile([128, QT, 1], mybir.dt.int32, tag="yb")
            yy = mids.tile([128, QT, 1], F32, tag="yy")
            t4 = mids.tile([128, QT, 1], F32, tag="t4")
            aa = mids.tile([128, QT, 1], F32, tag="aa")
            y2 = mids.tile([128, QT, 1], F32, tag="y2")
            nc.vector.tensor_scalar_add(vare[:], var[:], EPS)
            nc.vector.tensor_scalar_mul(xh[:], vare[:], 0.5)
            nc.vector.tensor_tensor(
                yb[:], vare[:].bitcast(mybir.dt.int32), one_i[:],
                op=mybir.AluOpType.arith_shift_right)
            nc.vector.tensor_tensor(
                yb[:], magic_i[:], yb[:], op=mybir.AluOpType.subtract)
            y0 = yb[:].bitcast(F32)
            for _ in range(2):
                nc.vector.tensor_mul(yy[:], y0, y0)
                nc.vector.tensor_mul(t4[:], xh[:], yy[:])
                nc.vector.tensor_scalar(
                    aa[:], t4[:], scalar1=-1.0, scalar2=1.5,
                    op0=mybir.AluOpType.mult, op1=mybir.AluOpType.add)
                nc.vector.tensor_mul(y2[:], y0, aa[:])
                nc.vector.tensor_copy(yb[:], y2[:].bitcast(mybir.dt.int32))
            nc.vector.tensor_copy(rs[:], y2[:])
            nc.vector.scalar_tensor_tensor(
                nb[:], msum[:], -0.5, rs[:],
                op0=mybir.AluOpType.mult, op1=mybir.AluOpType.mult,
            )
            for q in range(QT):
                nc.gpsimd.tensor_scalar(
                    o_s[:, q, :], ht_s[:, q, :],
                    scalar1=rs[:, q, :], scalar2=nb[:, q, :],
                    op0=mybir.AluOpType.mult, op1=mybir.AluOpType.add,
                )

            nc.sync.dma_start(out=out_ap, in_=o_s[:])

    return nc


def legalize_waits(nc, max_waits=1):
    """This toolchain's walrus allows only one sync wait per engine
    instruction.  Split extras onto same-engine NoOps inserted before the
    instruction (waiting earlier on the same engine is always safe)."""
    n_new = 0
    for f in nc.m.functions:
        for b in f.blocks:
            insts = b.instructions
            k = 0
            while k < len(insts):
                i = insts[k]
                si = i.sync_info
                if si is not None and len(si.on_wait) > max_waits:
                    waits = list(si.on_wait)
                    keep = waits[:max_waits]
                    extra = waits[max_waits:]
                    for w in extra:
                        nop = mybir.InstNoOp(name=f"wlg-{n_new}", ins=[],
                                             outs=[])
                        nop.engine = i.engine
                        nop.sync_info = mybir.SyncInfo(on_wait=[w],
                                                       on_update=[])
                        insts.insert(k, nop)
                        n_new += 1
                        k += 1
                    i.sync_info = mybir.SyncInfo(
                        on_wait=keep, on_update=list(si.on_update))
                k += 1
    return n_new


def get_built(loop_n=None, ablate=None):
    key = ("nc", loop_n, ablate)
    if key not in _BUILD_CACHE:
        nc = build_bass(loop_n, ablate)
        legalize_waits(nc)
        _BUILD_CACHE[key] = nc
    return _BUILD_CACHE[key]


def prep_inputs(x, h, W_a, W_g, b_a, b_g):
    wt0 = np.ascontiguousarray(
        np.concatenate([W_a[:, :128].T, W_g[:, :128].T],
                       axis=1)).astype(BF16_NP)
    wt1 = np.ascontiguousarray(
        np.concatenate([W_a[:, 128:].T, W_g[:, 128:].T],
                       axis=1)).astype(BF16_NP)
    bzg1 = np.concatenate([b_a, b_g])
    bzg = np.ascontiguousarray(
        np.concatenate([bzg1, bzg1])[None, :]).astype(BF16_NP)
    in_maps = []
    for c in range(N_CORES):
        r0 = c * ROWS
        in_maps.append({
            "hT": np.ascontiguousarray(h[r0:r0 + ROWS].T).astype(BF16_NP),
            "xT": np.ascontiguousarray(x[r0:r0 + ROWS].T).astype(BF16_NP),
            "wt0": wt0,
            "wt1": wt1,
            "bzg": bzg,
            "onesd": np.ones((1, 128), dtype=BF16_NP),
        })
    return in_maps


def run(in_maps, trace=False, loop_n=None):
    nc = get_built(loop_n)
    return run_bass_kernel_spmd(nc, in_maps, core_ids=list(range(N_CORES)),
                                trace=trace)


def kernel(x, h, W_a, W_g, b_a, b_g, gamma, beta):
    x = np.asarray(x, dtype=np.float32)
    h = np.asarray(h, dtype=np.float32)
    in_maps = prep_inputs(x, h, np.asarray(W_a), np.asarray(W_g),
                          np.asarray(b_a), np.asarray(b_g))
    res = run(in_maps, trace=False)
    parts = []
    for r in res.results:
        o = np.asarray(r["out"]).astype(np.float32)   # [N_SUP*128, SUP]
        # element (sti*128+p, q*128+f) is row sti*SUP + q*128 + p, feat f
        o = o.reshape(N_SUP, 128, RPS * T, 128).transpose(0, 2, 1, 3)
        parts.append(o.reshape(ROWS, 128))
    out = np.concatenate(parts, axis=0)
    gamma = np.asarray(gamma, dtype=np.float32)
    beta = np.asarray(beta, dtype=np.float32)
    if not (np.all(gamma == 1.0) and np.all(beta == 0.0)):
        out = out * gamma[None, :] + beta[None, :]
    return out
